# revision 12
# baseline (speedup 1.0000x reference)
"""Trainium2 Bass kernel for nn_AttentionBlock_2619930051209.

GQA sliding-window attention block: RMSNorm -> fused QKV -> YaRN RoPE ->
causal sliding-window (128) attention with learned sinks -> out-proj ->
residual.  T=2048, H=2048, NH=32, NKV=8, D=64.

Sharding (8 cores): tensor-parallel over KV-head groups.  Core c owns KV head
c and its 4 Q heads: 384 rows of qkv_w, 256 columns of out_w, sinks[4c:4c+4].
x is replicated; each core emits a partial [2048,2048] output (its heads'
contribution through out_w) and the host sums partials + residual + bias.

Device-side layout strategy:
  - x is DMA-transposed (xbar) from DRAM -> xT [h, t] tiles; QKV matmul runs
    with lhsT = xT tile (K=h), rhs = host-pretransposed qkv_w -> psum [t, f].
  - RMSNorm: ACT Square+accum on token-major x gives sumsq[t] per partition;
    rms_inv = exp(-0.5*ln(mean+eps)); applied as per-partition scale when
    copying the (un-normalized) QKV psum to SBUF.  norm_scale and the 1/8
    softmax scale are folded into the weights on the host.
  - RoPE via host tables (cos duplicated, sin with folded signs+swap) as 3
    strided DVE multiplies/adds per token tile.
  - Attention in S^T layout: S^T[k, q] = kT.T @ qT from PE-transposed q/k.
    exp on ACT, sliding-window mask as a 0/1 multiply, then AV with
    lhsT=expS^T slice, rhs=v augmented with a ones column so the softmax
    denominator comes out of the same matmul; per-partition reciprocal scale.
  - attn tiles PE-transposed to feature-major for the out-projection.
"""

import math
import os
import sys

import numpy as np
import ml_dtypes

for _p in ("/opt/trn_rl_repo", "/root/.axon_site/_ro/trn_rl_repo"):
    if os.path.isdir(_p) and _p not in sys.path:
        sys.path.insert(0, _p)

import concourse.bass as bass
import concourse.mybir as mybir
import concourse.tile as tile
from concourse.bass_utils import run_bass_kernel_spmd
from concourse.masks import make_identity

BF16 = ml_dtypes.bfloat16

# Problem constants (hardcoded; spec is fixed).
T = 2048
H = 2048
NH = 32
NKV = 8
D = 64
SW = 128
THETA = 150000.0
SF = 32.0
ALPHA = 1.0
BETA = 32.0
ICL = 4096
EPS = 1e-5
QM = NH // NKV          # 4 q heads per kv head
NCORES = 8
P = 128
TT = T // P             # 16 token tiles
HT = H // P             # 16 hidden tiles
F = QM * D + 2 * D      # 384 local qkv features (4 q heads + k + v)
QF = QM * D             # 256 local q features
SM_SCALE = 1.0 / math.sqrt(D)

_FP32 = mybir.dt.float32
_BF16 = mybir.dt.bfloat16

# ---------------------------------------------------------------------------
# This container's walrus build rejects instructions carrying more than one
# sync wait ("Too many sync wait commands", CoreV2GenImpl setupSyncWait), but
# Tile's scheduler freely attaches several.  Hoist all-but-one wait onto
# standalone EventSemaphore instructions on the same engine, placed directly
# before the owning instruction (sequencers execute in program order, so the
# semantics are identical).  Patching Bass.to_json_bytes covers every compile
# path (bass2jax / run_bass_kernel_spmd -> compile_bir_kernel).
# ---------------------------------------------------------------------------
_MAX_INLINE_WAITS = 1


def _split_sync_waits(bir_json: bytes) -> bytes:
    import json as _json

    bir = _json.loads(bir_json)
    for fn in bir.get("functions", []):
        for blk in fn.get("blocks", []):
            out = []
            for ins in blk["instructions"]:
                si = ins.get("sync_info")
                ow = (si or {}).get("on_wait") or []
                if len(ow) > _MAX_INLINE_WAITS:
                    keep = ow[-_MAX_INLINE_WAITS:]
                    for i, w in enumerate(ow[: -_MAX_INLINE_WAITS]):
                        carrier = {
                            "engine": ins["engine"],
                            "ins": [],
                            "outs": [],
                            "name": f"{ins['name']}-hw{i}",
                            "opcode": "EventSemaphore",
                            "sync_info": {"on_update": [], "on_wait": [w]},
                        }
                        if "debug" in ins:
                            carrier["debug"] = ins["debug"]
                        out.append(carrier)
                    si["on_wait"] = keep
                out.append(ins)
            blk["instructions"] = out
    return _json.dumps(bir).encode()


_orig_to_json_bytes = bass.Bass.to_json_bytes


def _patched_to_json_bytes(self):
    return _split_sync_waits(_orig_to_json_bytes(self))


bass.Bass.to_json_bytes = _patched_to_json_bytes


def _rope_cos_sin():
    """cos/sin [T, D/2] exactly as reference._compute_rope (fp64 -> fp32)."""
    freq = THETA ** (np.arange(0, D, 2, dtype=np.float64) / D)
    conc = 0.1 * math.log(SF) + 1.0
    d_half = D / 2
    low = d_half * math.log(ICL / (BETA * 2 * math.pi)) / math.log(THETA)
    high = d_half * math.log(ICL / (ALPHA * 2 * math.pi)) / math.log(THETA)
    interpolation = 1.0 / (SF * freq)
    extrapolation = 1.0 / freq
    ramp = (np.arange(d_half, dtype=np.float64) - low) / (high - low)
    m = 1.0 - np.clip(ramp, 0.0, 1.0)
    inv_freq = interpolation * (1.0 - m) + extrapolation * m
    t = np.arange(T, dtype=np.float64)
    freqs = np.outer(t, inv_freq)
    cos = (np.cos(freqs) * conc).astype(np.float32)
    sin = (np.sin(freqs) * conc).astype(np.float32)
    return cos, sin


def _build_tables():
    """Host-side constant tables shared by all cores."""
    cos, sin = _rope_cos_sin()  # [T, 32] fp32
    nrope = QM + 1  # 4 q heads + 1 k head get rope
    # COS table: per rope'd 64-block -> [cos | cos]
    cos64 = np.concatenate([cos, cos], axis=1)           # [T, 64]
    costab = np.tile(cos64, (1, nrope)).astype(BF16)     # [T, 320]
    # SIN table with signs folded + arranged for the swapped-half reads:
    #   tmp[:, blk 0:32]  = a2 * (-sin)   -> cols 0:32 hold -sin
    #   tmp[:, blk 32:64] = a1 * (+sin)   -> cols 32:64 hold +sin
    sin64 = np.concatenate([-sin, sin], axis=1)          # [T, 64]
    sinswtab = np.tile(sin64, (1, nrope)).astype(BF16)   # [T, 320]

    # Mask tile [128, 512]: two heads' [k=128, q=256] spans side by side.
    ki = np.arange(P)[:, None]
    qi = np.arange(P)[None, :]
    b0 = (ki <= qi).astype(np.float32)   # same k/q tile: causal upper-tri
    b1 = (ki > qi).astype(np.float32)    # q tile = k tile + 1: strict lower
    b = np.concatenate([b0, b1], axis=1)            # [128, 256]
    maskd = np.concatenate([b, b], axis=1).astype(BF16)  # [128, 512]
    return costab, sinswtab, maskd


def _build_program():
    nc = bass.Bass(use_seq_codegen=True)
    x_t = nc.dram_tensor("x", [T, H], _BF16, kind="ExternalInput")
    qkvw_t = nc.dram_tensor("qkvwT", [H, F], _BF16, kind="ExternalInput")
    outw_t = nc.dram_tensor("outwT", [QF, H], _BF16, kind="ExternalInput")
    cos_t = nc.dram_tensor("costab", [T, 5 * D], _BF16, kind="ExternalInput")
    sinsw_t = nc.dram_tensor("sinswtab", [T, 5 * D], _BF16, kind="ExternalInput")
    mask_t = nc.dram_tensor("maskD", [P, 4 * P], _BF16, kind="ExternalInput")
    esink_t = nc.dram_tensor("esink", [1, QM], _FP32, kind="ExternalInput")
    out_t = nc.dram_tensor("outp", [T, H], _BF16, kind="ExternalOutput")

    with tile.TileContext(nc) as tc:
        with (
            tc.tile_pool(name="singles", bufs=1) as singles,
            tc.tile_pool(name="xtok", bufs=3) as xtok_pool,
            tc.tile_pool(name="work", bufs=3) as work,
            tc.tile_pool(name="stats", bufs=4) as stats,
            tc.tile_pool(name="expm", bufs=6) as expm_pool,
            tc.tile_pool(name="outsb", bufs=3) as outsb_pool,
            tc.tile_pool(name="ps_big", bufs=5, space="PSUM") as ps_big,
            tc.tile_pool(name="ps_sm", bufs=3, space="PSUM") as ps_sm,
        ):
            # ---------------- constants into SBUF ----------------
            qkvw_sb = singles.tile([P, HT, F], _BF16)
            nc.sync.dma_start(
                out=qkvw_sb, in_=qkvw_t.rearrange("(a p) f -> p a f", p=P)
            )
            outw_sb = singles.tile([P, 2, H], _BF16)
            nc.sync.dma_start(
                out=outw_sb, in_=outw_t.rearrange("(a p) h -> p a h", p=P)
            )
            cos_sb = singles.tile([P, TT, 5 * D], _BF16)
            nc.sync.dma_start(
                out=cos_sb, in_=cos_t.rearrange("(a p) f -> p a f", p=P)
            )
            sinsw_sb = singles.tile([P, TT, 5 * D], _BF16)
            nc.sync.dma_start(
                out=sinsw_sb, in_=sinsw_t.rearrange("(a p) f -> p a f", p=P)
            )
            mask_sb = singles.tile([P, 4 * P], _BF16)
            nc.sync.dma_start(out=mask_sb, in_=mask_t[:, :])
            esink_sb = singles.tile([P, QM], _FP32)
            nc.gpsimd.dma_start(
                out=esink_sb,
                in_=bass.AP(
                    tensor=esink_t[:, :].tensor,
                    offset=esink_t[:, :].offset,
                    ap=[[0, P], [1, QM]],
                ),
            )
            ident_sb = singles.tile([P, P], _BF16)
            make_identity(nc, ident_sb)
            eps_sb = singles.tile([P, 1], _FP32)
            nc.vector.memset(eps_sb, EPS)

            # x transposed: xT[h, t] per h-tile, via xbar DMA transpose.
            xT_sb = singles.tile([P, HT, T], _BF16)
            for ht in range(HT):
                nc.sync.dma_start_transpose(
                    out=xT_sb[:, ht, :], in_=x_t[:, ht * P : (ht + 1) * P]
                )

            # Per-head q^T / k^T (feature-major), built tile by tile below.
            qT_sb = [
                singles.tile([D, T], _BF16, tag=f"qT{h}", name=f"qT{h}")
                for h in range(QM)
            ]
            kT_sb = singles.tile([D, T], _BF16)
            # v augmented with a ones column -> fused softmax denominator.
            vaug_sb = singles.tile([P, TT, D + 1], _BF16)
            nc.vector.memset(vaug_sb[:, :, D : D + 1], 1.0)
            # attn output, feature-major [f, t] for the out-projection.
            attnT_sb = singles.tile([P, 2, T], _BF16)
            # per-token-tile rms_inv columns
            rinv_sb = singles.tile([P, TT], _FP32)

            # ---------------- phase A: qkv + rope + transposes ----------------
            for tt in range(TT):
                tsl = slice(tt * P, (tt + 1) * P)
                # token-major x tile for the RMS statistic
                x_tok = xtok_pool.tile([P, H], _BF16)
                nc.gpsimd.dma_start(out=x_tok, in_=x_t[tsl, :])
                ssq = stats.tile([P, 1], _FP32)
                # sum over h of x^2 (ACT spline square, fp32 accumulate)
                nc.scalar.activation(
                    out=x_tok,
                    in_=x_tok,
                    func=mybir.ActivationFunctionType.Square,
                    accum_out=ssq,
                )
                # rms_inv = exp(-0.5 * ln(ssq/H + eps))
                lg = stats.tile([P, 1], _FP32)
                nc.scalar.activation(
                    out=lg,
                    in_=ssq,
                    func=mybir.ActivationFunctionType.Ln,
                    scale=1.0 / H,
                    bias=eps_sb,
                )
                nc.scalar.activation(
                    out=rinv_sb[:, tt : tt + 1],
                    in_=lg,
                    func=mybir.ActivationFunctionType.Exp,
                    scale=-0.5,
                )

                # QKV matmul: accumulate over h tiles -> psum [t, f]
                qkv_ps = ps_big.tile([P, F], _FP32, tag="ps")
                for ht in range(HT):
                    nc.tensor.matmul(
                        qkv_ps,
                        lhsT=xT_sb[:, ht, tsl],
                        rhs=qkvw_sb[:, ht, :],
                        start=(ht == 0),
                        stop=(ht == HT - 1),
                    )
                # normalize rows while copying out of PSUM
                qkv_sb = work.tile([P, F], _BF16, tag="qkv")
                nc.scalar.activation(
                    out=qkv_sb,
                    in_=qkv_ps,
                    func=mybir.ActivationFunctionType.Copy,
                    scale=rinv_sb[:, tt : tt + 1],
                )

                # RoPE on the first 320 features (4 q heads + k head)
                nr = 5 * D
                rsin = work.tile([P, nr], _BF16, tag="rsin")
                # swapped-half reads: a2 into first half slots, a1 into second
                a2 = qkv_sb[:, 0:nr].rearrange("p (h two d) -> p h two d", two=2, d=32)
                s_v = sinsw_sb[:, tt, :].rearrange("p (h two d) -> p h two d", two=2, d=32)
                r_v = rsin.rearrange("p (h two d) -> p h two d", two=2, d=32)
                nc.vector.tensor_mul(r_v[:, :, 0, :], a2[:, :, 1, :], s_v[:, :, 0, :])
                nc.vector.tensor_mul(r_v[:, :, 1, :], a2[:, :, 0, :], s_v[:, :, 1, :])
                rcos = work.tile([P, nr], _BF16, tag="rcos")
                nc.vector.tensor_mul(rcos, qkv_sb[:, 0:nr], cos_sb[:, tt, :])
                qkrot = work.tile([P, nr], _BF16, tag="qkrot")
                nc.vector.tensor_add(qkrot, rcos, rsin)

                # v (rms-scaled, no rope) into the augmented tile
                nc.vector.tensor_copy(vaug_sb[:, tt, 0:D], qkv_sb[:, 5 * D : 6 * D])

                # transpose each rope'd head block [128t, 64f] -> [64f, 128t]
                for hh in range(5):
                    tr_ps = ps_sm.tile([D, P], _BF16, tag="pss")
                    nc.tensor.transpose(
                        tr_ps, qkrot[:, hh * D : (hh + 1) * D], ident_sb
                    )
                    dst = qT_sb[hh] if hh < QM else kT_sb
                    nc.vector.tensor_copy(dst[:, tsl], tr_ps)

            # ---------------- phase B: attention + out-projection ----------------
            expm_tiles = [None] * TT
            for kb in range(TT):
                ksl = slice(kb * P, (kb + 1) * P)
                span = 2 * P if kb < TT - 1 else P
                pair = []
                for hp in range(2):
                    st_ps = ps_big.tile([P, 4 * P], _FP32, tag="ps")
                    for j in range(2):
                        h = 2 * hp + j
                        nc.tensor.matmul(
                            st_ps[:, j * 2 * P : j * 2 * P + span],
                            lhsT=kT_sb[:, ksl],
                            rhs=qT_sb[h][:, kb * P : kb * P + span],
                            start=True,
                            stop=True,
                        )
                    em = expm_pool.tile([P, 4 * P], _BF16, tag="expm")
                    if span == 2 * P:
                        ex = work.tile([P, 4 * P], _BF16, tag="exps")
                        nc.scalar.activation(
                            out=ex, in_=st_ps, func=mybir.ActivationFunctionType.Exp
                        )
                        nc.vector.tensor_mul(em, ex, mask_sb)
                    else:
                        ex = work.tile([P, 4 * P], _BF16, tag="exps")
                        for j in range(2):
                            c0 = j * 2 * P
                            nc.scalar.activation(
                                out=ex[:, c0 : c0 + P],
                                in_=st_ps[:, c0 : c0 + P],
                                func=mybir.ActivationFunctionType.Exp,
                            )
                            nc.vector.tensor_mul(
                                em[:, c0 : c0 + P],
                                ex[:, c0 : c0 + P],
                                mask_sb[:, c0 : c0 + P],
                            )
                    pair.append(em)
                expm_tiles[kb] = pair

                # AV + normalize for q tile qb == kb
                qb = kb
                qsl = slice(qb * P, (qb + 1) * P)
                attn_sb = work.tile([P, QF], _BF16, tag="attn")
                for h in range(QM):
                    hp, j = divmod(h, 2)
                    co = j * 2 * P
                    av_ps = ps_sm.tile([P, D + 1], _FP32, tag="pss")
                    if qb > 0:
                        nc.tensor.matmul(
                            av_ps,
                            lhsT=expm_tiles[qb - 1][hp][:, co + P : co + 2 * P],
                            rhs=vaug_sb[:, qb - 1, :],
                            start=True,
                            stop=False,
                        )
                    nc.tensor.matmul(
                        av_ps,
                        lhsT=expm_tiles[qb][hp][:, co : co + P],
                        rhs=vaug_sb[:, qb, :],
                        start=(qb == 0),
                        stop=True,
                    )
                    # denominator: l = sum(exp) + exp(sink_h); scale rows
                    l_sb = stats.tile([P, 1], _FP32, tag="l")
                    nc.vector.tensor_scalar_add(
                        l_sb, av_ps[:, D : D + 1], esink_sb[:, h : h + 1]
                    )
                    rl_sb = stats.tile([P, 1], _FP32, tag="rl")
                    nc.vector.reciprocal(rl_sb, l_sb)
                    nc.vector.tensor_scalar_mul(
                        attn_sb[:, h * D : (h + 1) * D], av_ps[:, 0:D], rl_sb
                    )

                # transpose attn [128t, 256f] -> feature-major columns
                for i in range(2):
                    tr_ps = ps_sm.tile([P, P], _BF16, tag="pss")
                    nc.tensor.transpose(
                        tr_ps, attn_sb[:, i * P : (i + 1) * P], ident_sb
                    )
                    nc.vector.tensor_copy(attnT_sb[:, i, qsl], tr_ps)

                # out-projection for this token tile: [t, H] partial
                out_sb = outsb_pool.tile([P, H], _BF16, tag="out")
                for ns in range(4):
                    nsl = slice(ns * 512, (ns + 1) * 512)
                    op_ps = ps_big.tile([P, 512], _FP32, tag="ps")
                    for i in range(2):
                        nc.tensor.matmul(
                            op_ps,
                            lhsT=attnT_sb[:, i, qsl],
                            rhs=outw_sb[:, i, nsl],
                            start=(i == 0),
                            stop=(i == 1),
                        )
                    if ns % 2 == 0:
                        nc.vector.tensor_copy(out_sb[:, nsl], op_ps)
                    else:
                        nc.scalar.copy(out_sb[:, nsl], op_ps)
                nc.sync.dma_start(out=out_t[qsl, :], in_=out_sb)

    return nc


def _run_pjrt_bench(nc, in_maps, iters):
    """Execute via PJRT like run_bass_via_pjrt, but without output-buffer
    donation so operands stay on device, and time pipelined repeat runs."""
    import time

    import jax
    import concourse.mybir as _mb
    from jax.sharding import Mesh, NamedSharding, PartitionSpec
    from jax.experimental.shard_map import shard_map
    from concourse import bass2jax

    bass2jax.install_neuronx_cc_hook()

    partition_name = nc.partition_id_tensor.name if nc.partition_id_tensor else None
    in_names, out_names, out_avals, zero_outs = [], [], [], []
    for alloc in nc.m.functions[0].allocations:
        if not isinstance(alloc, _mb.MemoryLocationSet):
            continue
        name = alloc.memorylocations[0].name
        if alloc.kind == "ExternalInput":
            if name != partition_name:
                in_names.append(name)
        elif alloc.kind == "ExternalOutput":
            shape = tuple(alloc.tensor_shape)
            dtype = _mb.dt.np(alloc.dtype)
            out_names.append(name)
            out_avals.append(jax.core.ShapedArray(shape, dtype))
            zero_outs.append(np.zeros(shape, dtype))
    n_params = len(in_names)
    all_in_names = list(in_names) + list(out_names)
    if partition_name is not None:
        all_in_names.append(partition_name)

    def _body(*args):
        operands = list(args)
        if partition_name is not None:
            operands.append(bass2jax.partition_id_tensor())
        outs = bass2jax._bass_exec_p.bind(
            *operands,
            out_avals=tuple(out_avals),
            in_names=tuple(all_in_names),
            out_names=tuple(out_names),
            lowering_input_output_aliases=(),
            sim_require_finite=True,
            sim_require_nnan=True,
            nc=nc,
        )
        return tuple(outs)

    devices = jax.devices()[:NCORES]
    mesh = Mesh(np.asarray(devices), ("core",))
    nouts = len(out_names)
    in_specs = (PartitionSpec("core"),) * (n_params + nouts)
    out_specs = (PartitionSpec("core"),) * nouts
    sharded = jax.jit(
        shard_map(
            _body, mesh=mesh, in_specs=in_specs, out_specs=out_specs, check_rep=False
        ),
        keep_unused=True,
    )
    per_core = [[np.asarray(m[name]) for name in in_names] for m in in_maps]
    sh = NamedSharding(mesh, PartitionSpec("core"))
    dev_in = [
        jax.device_put(
            np.concatenate([per_core[c][i] for c in range(NCORES)], axis=0), sh
        )
        for i in range(n_params)
    ]
    dev_zero = [
        jax.device_put(
            np.zeros((NCORES * z.shape[0], *z.shape[1:]), z.dtype), sh
        )
        for z in zero_outs
    ]

    out_arrs = sharded(*dev_in, *dev_zero)  # compile + first run
    jax.block_until_ready(out_arrs)

    # pipelined timing: dispatch all iters, block once
    t0 = time.time()
    last = None
    for _ in range(iters):
        last = sharded(*dev_in, *dev_zero)
    jax.block_until_ready(last)
    t1 = time.time()
    per_iter_ns = (t1 - t0) / iters * 1e9
    print(f"HW exec time: {per_iter_ns:.0f} ns")

    return [
        {
            name: np.asarray(out_arrs[i]).reshape(NCORES, *out_avals[i].shape)[c]
            for i, name in enumerate(out_names)
        }
        for c in range(NCORES)
    ]


_PROGRAM = None


def _get_program():
    global _PROGRAM
    if _PROGRAM is None:
        _PROGRAM = _build_program()
    return _PROGRAM


def kernel(x, norm_scale, sinks, qkv_w, qkv_b, out_w, out_b):
    x = np.asarray(x)
    norm_scale = np.asarray(norm_scale, dtype=np.float32)
    sinks = np.asarray(sinks)
    qkv_w = np.asarray(qkv_w)
    qkv_b = np.asarray(qkv_b)
    out_w = np.asarray(out_w)
    out_b = np.asarray(out_b)

    costab, sinswtab, maskd = _build_tables()
    nc = _get_program()

    # Fold norm_scale into all qkv_w columns; fold 1/sqrt(D) into q rows.
    w32 = qkv_w.astype(np.float32) * norm_scale[None, :]
    in_maps = []
    for c in range(NCORES):
        qrows = w32[c * QF : (c + 1) * QF] * SM_SCALE
        krows = w32[NH * D + c * D : NH * D + (c + 1) * D]
        vrows = w32[(NH + NKV) * D + c * D : (NH + NKV) * D + (c + 1) * D]
        wloc = np.concatenate([qrows, krows, vrows], axis=0)      # [384, H]
        qkvwT = np.ascontiguousarray(wloc.T).astype(BF16)         # [H, 384]
        outwT = np.ascontiguousarray(
            out_w[:, c * QF : (c + 1) * QF].astype(np.float32).T
        ).astype(BF16)                                            # [256, H]
        esink = np.exp(
            sinks[c * QM : (c + 1) * QM].astype(np.float32)
        ).reshape(1, QM)
        in_maps.append(
            {
                "x": x.astype(BF16),
                "qkvwT": qkvwT,
                "outwT": outwT,
                "costab": costab,
                "sinswtab": sinswtab,
                "maskD": maskd,
                "esink": esink,
            }
        )

    bench = int(os.environ.get("BASS_BENCH", "0") or "0")
    if bench:
        results = _run_pjrt_bench(nc, in_maps, bench)
    else:
        res = run_bass_kernel_spmd(nc, in_maps, core_ids=list(range(NCORES)))
        if res.exec_time_ns is not None:
            print(f"HW exec time: {res.exec_time_ns} ns")
        results = res.results

    acc = np.zeros((T, H), dtype=np.float32)
    for r in results:
        acc += r["outp"].astype(np.float32)
    acc += out_b.astype(np.float32)[None, :]
    acc += x.astype(np.float32)
    return acc.astype(x.dtype)


# revision 13
# speedup vs baseline: 7.5618x; 7.5618x over previous
"""Trainium2 Bass kernel for nn_AttentionBlock_2619930051209.

GQA sliding-window attention block: RMSNorm -> fused QKV -> YaRN RoPE ->
causal sliding-window (128) attention with learned sinks -> out-proj ->
residual.  T=2048, H=2048, NH=32, NKV=8, D=64.

Sharding (8 cores): tensor-parallel over KV-head groups.  Core c owns KV head
c and its 4 Q heads: 384 rows of qkv_w, 256 columns of out_w, sinks[4c:4c+4].
x is replicated; each core emits a partial [2048,2048] output (its heads'
contribution through out_w) and the host sums partials + residual + bias.

Device-side layout strategy:
  - x is DMA-transposed (xbar) from DRAM -> xT [h, t] tiles; QKV matmul runs
    with lhsT = xT tile (K=h), rhs = host-pretransposed qkv_w -> psum [t, f].
  - RMSNorm: ACT Square+accum on token-major x gives sumsq[t] per partition;
    rms_inv = exp(-0.5*ln(mean+eps)); applied as per-partition scale when
    copying the (un-normalized) QKV psum to SBUF.  norm_scale and the 1/8
    softmax scale are folded into the weights on the host.
  - RoPE via host tables (cos duplicated, sin with folded signs+swap) as 3
    strided DVE multiplies/adds per token tile.
  - Attention in S^T layout: S^T[k, q] = kT.T @ qT from PE-transposed q/k.
    exp on ACT, sliding-window mask as a 0/1 multiply, then AV with
    lhsT=expS^T slice, rhs=v augmented with a ones column so the softmax
    denominator comes out of the same matmul; per-partition reciprocal scale.
  - attn tiles PE-transposed to feature-major for the out-projection.
"""

import math
import os
import sys

import numpy as np
import ml_dtypes

for _p in ("/opt/trn_rl_repo", "/root/.axon_site/_ro/trn_rl_repo"):
    if os.path.isdir(_p) and _p not in sys.path:
        sys.path.insert(0, _p)

import concourse.bass as bass
import concourse.mybir as mybir
import concourse.tile as tile
from concourse.bass_utils import run_bass_kernel_spmd
from concourse.masks import make_identity

BF16 = ml_dtypes.bfloat16

# Problem constants (hardcoded; spec is fixed).
T = 2048
H = 2048
NH = 32
NKV = 8
D = 64
SW = 128
THETA = 150000.0
SF = 32.0
ALPHA = 1.0
BETA = 32.0
ICL = 4096
EPS = 1e-5
QM = NH // NKV          # 4 q heads per kv head
NCORES = 8
P = 128
TT = T // P             # 16 token tiles
HT = H // P             # 16 hidden tiles
F = QM * D + 2 * D      # 384 local qkv features (4 q heads + k + v)
QF = QM * D             # 256 local q features
SM_SCALE = 1.0 / math.sqrt(D)

_FP32 = mybir.dt.float32
_BF16 = mybir.dt.bfloat16

# ---------------------------------------------------------------------------
# This container's walrus build rejects instructions carrying more than one
# sync wait ("Too many sync wait commands", CoreV2GenImpl setupSyncWait), but
# Tile's scheduler freely attaches several.  Hoist all-but-one wait onto
# standalone EventSemaphore instructions on the same engine, placed directly
# before the owning instruction (sequencers execute in program order, so the
# semantics are identical).  Patching Bass.to_json_bytes covers every compile
# path (bass2jax / run_bass_kernel_spmd -> compile_bir_kernel).
# ---------------------------------------------------------------------------
_MAX_INLINE_WAITS = 1


def _split_sync_waits(bir_json: bytes) -> bytes:
    import json as _json

    bir = _json.loads(bir_json)
    for fn in bir.get("functions", []):
        for blk in fn.get("blocks", []):
            out = []
            for ins in blk["instructions"]:
                si = ins.get("sync_info")
                ow = (si or {}).get("on_wait") or []
                if len(ow) > _MAX_INLINE_WAITS:
                    keep = ow[-_MAX_INLINE_WAITS:]
                    for i, w in enumerate(ow[: -_MAX_INLINE_WAITS]):
                        carrier = {
                            "engine": ins["engine"],
                            "ins": [],
                            "outs": [],
                            "name": f"{ins['name']}-hw{i}",
                            "opcode": "EventSemaphore",
                            "sync_info": {"on_update": [], "on_wait": [w]},
                        }
                        if "debug" in ins:
                            carrier["debug"] = ins["debug"]
                        out.append(carrier)
                    si["on_wait"] = keep
                out.append(ins)
            blk["instructions"] = out
    return _json.dumps(bir).encode()


_orig_to_json_bytes = bass.Bass.to_json_bytes


def _patched_to_json_bytes(self):
    return _split_sync_waits(_orig_to_json_bytes(self))


bass.Bass.to_json_bytes = _patched_to_json_bytes


def _rope_cos_sin():
    """cos/sin [T, D/2] exactly as reference._compute_rope (fp64 -> fp32)."""
    freq = THETA ** (np.arange(0, D, 2, dtype=np.float64) / D)
    conc = 0.1 * math.log(SF) + 1.0
    d_half = D / 2
    low = d_half * math.log(ICL / (BETA * 2 * math.pi)) / math.log(THETA)
    high = d_half * math.log(ICL / (ALPHA * 2 * math.pi)) / math.log(THETA)
    interpolation = 1.0 / (SF * freq)
    extrapolation = 1.0 / freq
    ramp = (np.arange(d_half, dtype=np.float64) - low) / (high - low)
    m = 1.0 - np.clip(ramp, 0.0, 1.0)
    inv_freq = interpolation * (1.0 - m) + extrapolation * m
    t = np.arange(T, dtype=np.float64)
    freqs = np.outer(t, inv_freq)
    cos = (np.cos(freqs) * conc).astype(np.float32)
    sin = (np.sin(freqs) * conc).astype(np.float32)
    return cos, sin


def _build_tables():
    """Host-side constant tables shared by all cores."""
    cos, sin = _rope_cos_sin()  # [T, 32] fp32
    nrope = QM + 1  # 4 q heads + 1 k head get rope
    # COS table: per rope'd 64-block -> [cos | cos]
    cos64 = np.concatenate([cos, cos], axis=1)           # [T, 64]
    costab = np.tile(cos64, (1, nrope)).astype(BF16)     # [T, 320]
    # SIN table with signs folded + arranged for the swapped-half reads:
    #   tmp[:, blk 0:32]  = a2 * (-sin)   -> cols 0:32 hold -sin
    #   tmp[:, blk 32:64] = a1 * (+sin)   -> cols 32:64 hold +sin
    sin64 = np.concatenate([-sin, sin], axis=1)          # [T, 64]
    sinswtab = np.tile(sin64, (1, nrope)).astype(BF16)   # [T, 320]

    # Mask tile [128, 512]: two heads' [k=128, q=256] spans side by side.
    ki = np.arange(P)[:, None]
    qi = np.arange(P)[None, :]
    b0 = (ki <= qi).astype(np.float32)   # same k/q tile: causal upper-tri
    b1 = (ki > qi).astype(np.float32)    # q tile = k tile + 1: strict lower
    b = np.concatenate([b0, b1], axis=1)            # [128, 256]
    maskd = np.concatenate([b, b], axis=1).astype(BF16)  # [128, 512]
    return costab, sinswtab, maskd


def _build_program():
    nc = bass.Bass(use_seq_codegen=True)
    x_t = nc.dram_tensor("x", [T, H], _BF16, kind="ExternalInput")
    qkvw_t = nc.dram_tensor("qkvwT", [H, F], _BF16, kind="ExternalInput")
    outw_t = nc.dram_tensor("outwT", [QF, H], _BF16, kind="ExternalInput")
    cos_t = nc.dram_tensor("costab", [T, 5 * D], _BF16, kind="ExternalInput")
    sinsw_t = nc.dram_tensor("sinswtab", [T, 5 * D], _BF16, kind="ExternalInput")
    mask_t = nc.dram_tensor("maskD", [P, 4 * P], _BF16, kind="ExternalInput")
    esink_t = nc.dram_tensor("esink", [1, QM], _FP32, kind="ExternalInput")
    out_t = nc.dram_tensor("outp", [T, H], _BF16, kind="ExternalOutput")

    with tile.TileContext(nc) as tc:
        with (
            tc.tile_pool(name="singles", bufs=1) as singles,
            tc.tile_pool(name="xtok", bufs=3) as xtok_pool,
            tc.tile_pool(name="work", bufs=3) as work,
            tc.tile_pool(name="stats", bufs=4) as stats,
            tc.tile_pool(name="expm", bufs=6) as expm_pool,
            tc.tile_pool(name="outsb", bufs=3) as outsb_pool,
            tc.tile_pool(name="ps_big", bufs=5, space="PSUM") as ps_big,
            tc.tile_pool(name="ps_sm", bufs=3, space="PSUM") as ps_sm,
        ):
            # ---------------- constants into SBUF ----------------
            qkvw_sb = singles.tile([P, HT, F], _BF16)
            nc.sync.dma_start(
                out=qkvw_sb, in_=qkvw_t.rearrange("(a p) f -> p a f", p=P)
            )
            outw_sb = singles.tile([P, 2, H], _BF16)
            nc.sync.dma_start(
                out=outw_sb, in_=outw_t.rearrange("(a p) h -> p a h", p=P)
            )
            cos_sb = singles.tile([P, TT, 5 * D], _BF16)
            nc.sync.dma_start(
                out=cos_sb, in_=cos_t.rearrange("(a p) f -> p a f", p=P)
            )
            sinsw_sb = singles.tile([P, TT, 5 * D], _BF16)
            nc.sync.dma_start(
                out=sinsw_sb, in_=sinsw_t.rearrange("(a p) f -> p a f", p=P)
            )
            mask_sb = singles.tile([P, 4 * P], _BF16)
            nc.sync.dma_start(out=mask_sb, in_=mask_t[:, :])
            esink_sb = singles.tile([P, QM], _FP32)
            nc.gpsimd.dma_start(
                out=esink_sb,
                in_=bass.AP(
                    tensor=esink_t[:, :].tensor,
                    offset=esink_t[:, :].offset,
                    ap=[[0, P], [1, QM]],
                ),
            )
            ident_sb = singles.tile([P, P], _BF16)
            make_identity(nc, ident_sb)
            eps_sb = singles.tile([P, 1], _FP32)
            nc.vector.memset(eps_sb, EPS)

            # x transposed: xT[h, t] per h-tile, via xbar DMA transpose.
            xT_sb = singles.tile([P, HT, T], _BF16)
            for ht in range(HT):
                nc.sync.dma_start_transpose(
                    out=xT_sb[:, ht, :], in_=x_t[:, ht * P : (ht + 1) * P]
                )

            # Per-head q^T / k^T (feature-major), built tile by tile below.
            qT_sb = [
                singles.tile([D, T], _BF16, tag=f"qT{h}", name=f"qT{h}")
                for h in range(QM)
            ]
            kT_sb = singles.tile([D, T], _BF16)
            # v augmented with a ones column -> fused softmax denominator.
            vaug_sb = singles.tile([P, TT, D + 1], _BF16)
            nc.vector.memset(vaug_sb[:, :, D : D + 1], 1.0)
            # attn output, feature-major [f, t] for the out-projection.
            attnT_sb = singles.tile([P, 2, T], _BF16)
            # per-token-tile rms_inv columns
            rinv_sb = singles.tile([P, TT], _FP32)

            # ---------------- phase A: qkv + rope + transposes ----------------
            for tt in range(TT):
                tsl = slice(tt * P, (tt + 1) * P)
                # token-major x tile for the RMS statistic
                x_tok = xtok_pool.tile([P, H], _BF16)
                nc.gpsimd.dma_start(out=x_tok, in_=x_t[tsl, :])
                ssq = stats.tile([P, 1], _FP32)
                # sum over h of x^2 (ACT spline square, fp32 accumulate)
                nc.scalar.activation(
                    out=x_tok,
                    in_=x_tok,
                    func=mybir.ActivationFunctionType.Square,
                    accum_out=ssq,
                )
                # rms_inv = exp(-0.5 * ln(ssq/H + eps))
                lg = stats.tile([P, 1], _FP32)
                nc.scalar.activation(
                    out=lg,
                    in_=ssq,
                    func=mybir.ActivationFunctionType.Ln,
                    scale=1.0 / H,
                    bias=eps_sb,
                )
                nc.scalar.activation(
                    out=rinv_sb[:, tt : tt + 1],
                    in_=lg,
                    func=mybir.ActivationFunctionType.Exp,
                    scale=-0.5,
                )

                # QKV matmul: accumulate over h tiles -> psum [t, f]
                qkv_ps = ps_big.tile([P, F], _FP32, tag="ps")
                for ht in range(HT):
                    nc.tensor.matmul(
                        qkv_ps,
                        lhsT=xT_sb[:, ht, tsl],
                        rhs=qkvw_sb[:, ht, :],
                        start=(ht == 0),
                        stop=(ht == HT - 1),
                    )
                # normalize rows while copying out of PSUM
                qkv_sb = work.tile([P, F], _BF16, tag="qkv")
                nc.scalar.activation(
                    out=qkv_sb,
                    in_=qkv_ps,
                    func=mybir.ActivationFunctionType.Copy,
                    scale=rinv_sb[:, tt : tt + 1],
                )

                # RoPE on the first 320 features (4 q heads + k head)
                nr = 5 * D
                rsin = work.tile([P, nr], _BF16, tag="rsin")
                # swapped-half reads: a2 into first half slots, a1 into second
                a2 = qkv_sb[:, 0:nr].rearrange("p (h two d) -> p h two d", two=2, d=32)
                s_v = sinsw_sb[:, tt, :].rearrange("p (h two d) -> p h two d", two=2, d=32)
                r_v = rsin.rearrange("p (h two d) -> p h two d", two=2, d=32)
                nc.vector.tensor_mul(r_v[:, :, 0, :], a2[:, :, 1, :], s_v[:, :, 0, :])
                nc.vector.tensor_mul(r_v[:, :, 1, :], a2[:, :, 0, :], s_v[:, :, 1, :])
                rcos = work.tile([P, nr], _BF16, tag="rcos")
                nc.vector.tensor_mul(rcos, qkv_sb[:, 0:nr], cos_sb[:, tt, :])
                qkrot = work.tile([P, nr], _BF16, tag="qkrot")
                nc.vector.tensor_add(qkrot, rcos, rsin)

                # v (rms-scaled, no rope) into the augmented tile
                nc.vector.tensor_copy(vaug_sb[:, tt, 0:D], qkv_sb[:, 5 * D : 6 * D])

                # transpose each rope'd head block [128t, 64f] -> [64f, 128t]
                for hh in range(5):
                    tr_ps = ps_sm.tile([D, P], _BF16, tag="pss")
                    nc.tensor.transpose(
                        tr_ps, qkrot[:, hh * D : (hh + 1) * D], ident_sb
                    )
                    dst = qT_sb[hh] if hh < QM else kT_sb
                    nc.vector.tensor_copy(dst[:, tsl], tr_ps)

            # ---------------- phase B: attention + out-projection ----------------
            expm_tiles = [None] * TT
            for kb in range(TT):
                ksl = slice(kb * P, (kb + 1) * P)
                span = 2 * P if kb < TT - 1 else P
                pair = []
                for hp in range(2):
                    st_ps = ps_big.tile([P, 4 * P], _FP32, tag="ps")
                    for j in range(2):
                        h = 2 * hp + j
                        nc.tensor.matmul(
                            st_ps[:, j * 2 * P : j * 2 * P + span],
                            lhsT=kT_sb[:, ksl],
                            rhs=qT_sb[h][:, kb * P : kb * P + span],
                            start=True,
                            stop=True,
                        )
                    em = expm_pool.tile([P, 4 * P], _BF16, tag="expm")
                    if span == 2 * P:
                        ex = work.tile([P, 4 * P], _BF16, tag="exps")
                        nc.scalar.activation(
                            out=ex, in_=st_ps, func=mybir.ActivationFunctionType.Exp
                        )
                        nc.vector.tensor_mul(em, ex, mask_sb)
                    else:
                        ex = work.tile([P, 4 * P], _BF16, tag="exps")
                        for j in range(2):
                            c0 = j * 2 * P
                            nc.scalar.activation(
                                out=ex[:, c0 : c0 + P],
                                in_=st_ps[:, c0 : c0 + P],
                                func=mybir.ActivationFunctionType.Exp,
                            )
                            nc.vector.tensor_mul(
                                em[:, c0 : c0 + P],
                                ex[:, c0 : c0 + P],
                                mask_sb[:, c0 : c0 + P],
                            )
                    pair.append(em)
                expm_tiles[kb] = pair

                # AV + normalize for q tile qb == kb
                qb = kb
                qsl = slice(qb * P, (qb + 1) * P)
                attn_sb = work.tile([P, QF], _BF16, tag="attn")
                for h in range(QM):
                    hp, j = divmod(h, 2)
                    co = j * 2 * P
                    av_ps = ps_sm.tile([P, D + 1], _FP32, tag="pss")
                    if qb > 0:
                        nc.tensor.matmul(
                            av_ps,
                            lhsT=expm_tiles[qb - 1][hp][:, co + P : co + 2 * P],
                            rhs=vaug_sb[:, qb - 1, :],
                            start=True,
                            stop=False,
                        )
                    nc.tensor.matmul(
                        av_ps,
                        lhsT=expm_tiles[qb][hp][:, co : co + P],
                        rhs=vaug_sb[:, qb, :],
                        start=(qb == 0),
                        stop=True,
                    )
                    # denominator: l = sum(exp) + exp(sink_h); scale rows
                    l_sb = stats.tile([P, 1], _FP32, tag="l")
                    nc.vector.tensor_scalar_add(
                        l_sb, av_ps[:, D : D + 1], esink_sb[:, h : h + 1]
                    )
                    rl_sb = stats.tile([P, 1], _FP32, tag="rl")
                    nc.vector.reciprocal(rl_sb, l_sb)
                    nc.vector.tensor_scalar_mul(
                        attn_sb[:, h * D : (h + 1) * D], av_ps[:, 0:D], rl_sb
                    )

                # transpose attn [128t, 256f] -> feature-major columns
                for i in range(2):
                    tr_ps = ps_sm.tile([P, P], _BF16, tag="pss")
                    nc.tensor.transpose(
                        tr_ps, attn_sb[:, i * P : (i + 1) * P], ident_sb
                    )
                    nc.vector.tensor_copy(attnT_sb[:, i, qsl], tr_ps)

                # out-projection for this token tile: [t, H] partial
                out_sb = outsb_pool.tile([P, H], _BF16, tag="out")
                for ns in range(4):
                    nsl = slice(ns * 512, (ns + 1) * 512)
                    op_ps = ps_big.tile([P, 512], _FP32, tag="ps")
                    for i in range(2):
                        nc.tensor.matmul(
                            op_ps,
                            lhsT=attnT_sb[:, i, qsl],
                            rhs=outw_sb[:, i, nsl],
                            start=(i == 0),
                            stop=(i == 1),
                        )
                    if ns % 2 == 0:
                        nc.vector.tensor_copy(out_sb[:, nsl], op_ps)
                    else:
                        nc.scalar.copy(out_sb[:, nsl], op_ps)
                nc.sync.dma_start(out=out_t[qsl, :], in_=out_sb)

    return nc


def _run_pjrt_bench(nc, in_maps, iters):
    """Execute via PJRT like run_bass_via_pjrt, but without output-buffer
    donation so operands stay on device, and time pipelined repeat runs."""
    import time

    import jax
    import concourse.mybir as _mb
    from jax.sharding import Mesh, NamedSharding, PartitionSpec
    from jax.experimental.shard_map import shard_map
    from concourse import bass2jax

    bass2jax.install_neuronx_cc_hook()

    partition_name = nc.partition_id_tensor.name if nc.partition_id_tensor else None
    in_names, out_names, out_avals, zero_outs = [], [], [], []
    for alloc in nc.m.functions[0].allocations:
        if not isinstance(alloc, _mb.MemoryLocationSet):
            continue
        name = alloc.memorylocations[0].name
        if alloc.kind == "ExternalInput":
            if name != partition_name:
                in_names.append(name)
        elif alloc.kind == "ExternalOutput":
            shape = tuple(alloc.tensor_shape)
            dtype = _mb.dt.np(alloc.dtype)
            out_names.append(name)
            out_avals.append(jax.core.ShapedArray(shape, dtype))
            zero_outs.append(np.zeros(shape, dtype))
    n_params = len(in_names)
    all_in_names = list(in_names) + list(out_names)
    if partition_name is not None:
        all_in_names.append(partition_name)

    def _body(*args):
        operands = list(args)
        if partition_name is not None:
            operands.append(bass2jax.partition_id_tensor())
        outs = bass2jax._bass_exec_p.bind(
            *operands,
            out_avals=tuple(out_avals),
            in_names=tuple(all_in_names),
            out_names=tuple(out_names),
            lowering_input_output_aliases=(),
            sim_require_finite=True,
            sim_require_nnan=True,
            nc=nc,
        )
        return tuple(outs)

    devices = jax.devices()[:NCORES]
    mesh = Mesh(np.asarray(devices), ("core",))
    nouts = len(out_names)
    in_specs = (PartitionSpec("core"),) * (n_params + nouts)
    out_specs = (PartitionSpec("core"),) * nouts
    sharded = jax.jit(
        shard_map(
            _body, mesh=mesh, in_specs=in_specs, out_specs=out_specs, check_rep=False
        ),
        keep_unused=True,
    )
    per_core = [[np.asarray(m[name]) for name in in_names] for m in in_maps]
    sh = NamedSharding(mesh, PartitionSpec("core"))
    dev_in = [
        jax.device_put(
            np.concatenate([per_core[c][i] for c in range(NCORES)], axis=0), sh
        )
        for i in range(n_params)
    ]
    dev_zero = [
        jax.device_put(
            np.zeros((NCORES * z.shape[0], *z.shape[1:]), z.dtype), sh
        )
        for z in zero_outs
    ]

    out_arrs = sharded(*dev_in, *dev_zero)  # compile + first run
    jax.block_until_ready(out_arrs)

    # pipelined timing: dispatch all iters, block once
    t0 = time.time()
    last = None
    for _ in range(iters):
        last = sharded(*dev_in, *dev_zero)
    jax.block_until_ready(last)
    t1 = time.time()
    per_iter_ns = (t1 - t0) / iters * 1e9
    print(f"HW exec time: {per_iter_ns:.0f} ns")

    return [
        {
            name: np.asarray(out_arrs[i]).reshape(NCORES, *out_avals[i].shape)[c]
            for i, name in enumerate(out_names)
        }
        for c in range(NCORES)
    ]


_PROGRAM = None


def _get_program():
    global _PROGRAM
    if _PROGRAM is None:
        _PROGRAM = _build_program()
    return _PROGRAM


def kernel(x, norm_scale, sinks, qkv_w, qkv_b, out_w, out_b):
    x = np.asarray(x)
    norm_scale = np.asarray(norm_scale, dtype=np.float32)
    sinks = np.asarray(sinks)
    qkv_w = np.asarray(qkv_w)
    qkv_b = np.asarray(qkv_b)
    out_w = np.asarray(out_w)
    out_b = np.asarray(out_b)

    costab, sinswtab, maskd = _build_tables()
    nc = _get_program()

    # Fold norm_scale into all qkv_w columns; fold 1/sqrt(D) into q rows.
    w32 = qkv_w.astype(np.float32) * norm_scale[None, :]
    in_maps = []
    for c in range(NCORES):
        qrows = w32[c * QF : (c + 1) * QF] * SM_SCALE
        krows = w32[NH * D + c * D : NH * D + (c + 1) * D]
        vrows = w32[(NH + NKV) * D + c * D : (NH + NKV) * D + (c + 1) * D]
        wloc = np.concatenate([qrows, krows, vrows], axis=0)      # [384, H]
        qkvwT = np.ascontiguousarray(wloc.T).astype(BF16)         # [H, 384]
        outwT = np.ascontiguousarray(
            out_w[:, c * QF : (c + 1) * QF].astype(np.float32).T
        ).astype(BF16)                                            # [256, H]
        esink = np.exp(
            sinks[c * QM : (c + 1) * QM].astype(np.float32)
        ).reshape(1, QM)
        in_maps.append(
            {
                "x": x.astype(BF16),
                "qkvwT": qkvwT,
                "outwT": outwT,
                "costab": costab,
                "sinswtab": sinswtab,
                "maskD": maskd,
                "esink": esink,
            }
        )

    bench = int(os.environ.get("BASS_BENCH", "0") or "0")
    if bench:
        results = _run_pjrt_bench(nc, in_maps, bench)
    else:
        kw = {}
        if os.environ.get("BASS_TRACE"):
            import tempfile

            kw["tmpdir"] = tempfile.mkdtemp(prefix="trace_", dir="/tmp")
            print(f"trace tmpdir: {kw['tmpdir']}")
        res = run_bass_kernel_spmd(nc, in_maps, core_ids=list(range(NCORES)), **kw)
        if res.exec_time_ns is not None:
            print(f"HW exec time: {res.exec_time_ns} ns")
        if res.instructions_and_trace is not None:
            print(f"trace path: {res.instructions_and_trace[1]}")
        results = res.results

    acc = np.zeros((T, H), dtype=np.float32)
    for r in results:
        acc += r["outp"].astype(np.float32)
    acc += out_b.astype(np.float32)[None, :]
    acc += x.astype(np.float32)
    return acc.astype(x.dtype)
